# revision 23
# baseline (speedup 1.0000x reference)
"""DeepseekV3 MLA attention (B=1, S=2048, D=2048, H=16) on 8 trn2 NeuronCores.

v3 strategy:
  - stage 1 (q_a / kv_a + rmsnorm + rope) AND q_b (for ALL 16 heads) are
    SEQUENCE-SHARDED: core c computes them only for its 256-token chunk;
  - kv activations (ckvn 4 tiles + kper 1 tile, [128,256] bf16) are
    exchanged with an early HBM AllGather (0.33 MB in -> 2.6 MB out) that
    overlaps the q_a/q_b compute;
  - q heads are exchanged with an AllToAll (1.57 MB): core c sends, for each
    destination j, [qn(2j) | qn(2j+1) | qpe-pair(j)] on its local tokens;
  - stage 2 (kv_b, causal flash attention, o_proj slice for 2 owned heads)
    is tensor-parallel over heads; host sums bf16 partials.

RoPE: deinterleave folded into weights; rotate-half partner produced by a
32-row partition-swap DMA with the sign folded into the sin table. rmsnorm
inv scale folded into the PSUM->SBUF copies after q_b (per-token scalar
commutes through the linear map).

All weights shipped in partition-major tiled layout [128, ktiles*cols] so
each SBUF weight load is one DMA.
"""

import numpy as np
import ml_dtypes

import concourse.bass as bass
import concourse.mybir as mybir
import concourse.tile as tile
from concourse.bass_utils import run_bass_kernel_spmd

BF16 = ml_dtypes.bfloat16
F32 = mybir.dt.float32
BF = mybir.dt.bfloat16

B, S, D = 1, 2048, 2048
H = 16
N_CORES = 8
HPC = H // N_CORES  # heads per core = 2
Q_LORA = 1536
KV_LORA = 512
NOPE = 128
ROPE = 64
VD = 128
QHD = NOPE + ROPE  # 192
THETA = 50000.0
EPS = 1e-6
SCALE = QHD ** (-0.5)

LOC = S // N_CORES   # 256 local chunk
NQ = 512             # q-chunk (matmul free dim) in stage 2
NCHUNK = S // NQ     # 4
KT = S // 128        # 16 k-tiles
QL_T = Q_LORA // 128  # 12
D_T = D // 128        # 16
CV_T = KV_LORA // 128  # 4
QB_T = H + N_CORES    # 24 q_b output tiles: 16 nope + 8 pe-pairs
AF = mybir.ActivationFunctionType

LAST_RESULTS = None
_CACHE = {}


def _tiled(a, rows=128):
    """[kt*rows, cols] -> partition-major [rows, kt*cols] (single-DMA load)."""
    kt = a.shape[0] // rows
    return np.ascontiguousarray(
        a.reshape(kt, rows, a.shape[1]).transpose(1, 0, 2).reshape(rows, -1)
    )


# ----------------------------------------------------------------------------
# host-side weight preparation
# ----------------------------------------------------------------------------

def _deint_perm():
    p = np.empty(ROPE, dtype=np.int64)
    p[:32] = 2 * np.arange(32)
    p[32:] = 2 * np.arange(32) + 1
    return p


def _rope_tables(position_ids):
    pos = np.asarray(position_ids).reshape(-1).astype(np.float32)  # [S]
    inv_freq = (1.0 / (THETA ** (np.arange(0, ROPE, 2, dtype=np.float32) / ROPE)))
    freqs = np.outer(pos, inv_freq)  # [S, 32]
    cos32 = np.cos(freqs).T.astype(np.float32)  # [32, S]
    sin32 = np.sin(freqs).T.astype(np.float32)
    cos128 = np.tile(cos32, (4, 1))  # [128, S]
    sin128 = np.tile(sin32, (4, 1))
    # rotate-half sign folded into sin: row j multiplies the swapped partner,
    # with sign -1 for j%64 < 32
    sgn = np.where((np.arange(128) % 64) < 32, -1.0, 1.0).astype(np.float32)
    sin128s = sgn[:, None] * sin128
    return cos128, sin128s


def _causal_mask_big():
    dk = np.arange(128)[:, None]
    u = np.arange(1024)[None, :]
    return (u >= dk + 384).astype(BF16)


def _prep_inputs(inputs):
    hidden = np.asarray(inputs["hidden_states"], dtype=np.float32)[0]  # [S, D]
    position_ids = np.asarray(inputs["position_ids"])
    q_a_w = np.asarray(inputs["q_a_w"], dtype=np.float32)
    q_a_ln_w = np.asarray(inputs["q_a_ln_w"], dtype=np.float32)
    q_b_w = np.asarray(inputs["q_b_w"], dtype=np.float32)
    kv_a_w = np.asarray(inputs["kv_a_w"], dtype=np.float32)
    kv_a_ln_w = np.asarray(inputs["kv_a_ln_w"], dtype=np.float32)
    kv_b_w = np.asarray(inputs["kv_b_w"], dtype=np.float32)
    o_w = np.asarray(inputs["o_w"], dtype=np.float32)

    dp = _deint_perm()
    dps = dp[(np.arange(ROPE) ^ 32)]

    hT = np.ascontiguousarray(hidden.T).astype(BF16)  # [D, S]

    shared = {}
    shared["qaT"] = _tiled(np.ascontiguousarray(q_a_w.T).astype(BF16))
    kva_cols = np.concatenate(
        [kv_a_w[:KV_LORA], kv_a_w[KV_LORA + dp], kv_a_w[KV_LORA + dps]], axis=0
    )  # [640, D]
    shared["kvaT"] = _tiled(np.ascontiguousarray(kva_cols.T).astype(BF16))

    # q_b for ALL heads: [16 nope tiles | 8 pe-pair tiles] x 1536
    qb = (q_b_w * q_a_ln_w[None, :] * SCALE).reshape(H, QHD, Q_LORA)
    rows = [qb[h, :NOPE] for h in range(H)]
    for j in range(N_CORES):
        rows.append(
            np.concatenate([qb[2 * j, NOPE + dp], qb[2 * j + 1, NOPE + dp]], axis=0)
        )
    qball = np.concatenate(rows, axis=0)  # [24*128, 1536]
    shared["qbAllT"] = _tiled(np.ascontiguousarray(qball.T).astype(BF16))  # [128, 12*3072]

    cos128, sin128s = _rope_tables(position_ids)
    shared["maskb"] = _causal_mask_big()

    kvb = (kv_b_w * kv_a_ln_w[None, :]).reshape(H, NOPE + VD, KV_LORA)

    per_core = []
    for c in range(N_CORES):
        h0, h1 = HPC * c, HPC * c + 1
        kb_cols = np.concatenate([kvb[h0, :NOPE], kvb[h1, :NOPE]], axis=0)
        vb_cols = np.concatenate([kvb[h0, NOPE:], kvb[h1, NOPE:]], axis=0)
        o_slice = o_w[:, VD * h0 : VD * (h1 + 1)]
        cl = cos128[:, LOC * c : LOC * (c + 1)]  # [128, 256]
        sl = sin128s[:, LOC * c : LOC * (c + 1)]
        per_core.append(
            {
                "hTloc": _tiled(np.ascontiguousarray(hT[:, LOC * c : LOC * (c + 1)])),
                "cosl": np.ascontiguousarray(cl).astype(BF16),  # [128, 256]
                "sinl": np.ascontiguousarray(sl).astype(BF16),
                "kbT": _tiled(np.ascontiguousarray(kb_cols.T).astype(BF16)),
                "vbT": _tiled(np.ascontiguousarray(vb_cols.T).astype(BF16)),
                "owT": _tiled(np.ascontiguousarray(o_slice.T).astype(BF16)),
            }
        )
    return shared, per_core


# ----------------------------------------------------------------------------
# numpy simulation of the device program (for host-side validation)
# ----------------------------------------------------------------------------

def _untile(a, kt):
    return a.reshape(128, kt, -1).transpose(1, 0, 2).reshape(128 * kt, -1)


def _sim_stage1(shared, pc):
    """One core's stage 1+q_b on its local chunk.

    Returns (qn [16][128,256], qpe [8 pairs][128,256], ckvn, kperB) bf16."""
    bf = lambda x: x.astype(BF16).astype(np.float32)
    hT = _untile(pc["hTloc"], D_T).astype(np.float32)
    qaT = _untile(shared["qaT"], D_T).astype(np.float32)
    kvaT = _untile(shared["kvaT"], D_T).astype(np.float32)
    qbAll = _untile(shared["qbAllT"], QL_T).astype(np.float32)  # [1536, 3072]
    cosl = pc["cosl"].astype(np.float32)
    sinl = pc["sinl"].astype(np.float32)

    ckvT = kvaT.T @ hT
    ckv = ckvT[:KV_LORA]
    ckvb = bf(ckv)
    ssc = (bf(ckvb * ckvb)).sum(axis=0)
    invc = 1.0 / np.sqrt(ssc / KV_LORA + EPS)
    ckvn = bf(ckvb * invc)
    kpe, kpe2 = ckvT[512:576], ckvT[576:640]
    kper = bf(kpe * cosl[0:64] + kpe2 * sinl[0:64])
    kperB = np.concatenate([kper, kper], axis=0)

    qaTx = qaT.T @ hT
    qab = bf(qaTx)
    ssq = (bf(qab * qab)).sum(axis=0)
    inv = 1.0 / np.sqrt(ssq / Q_LORA + EPS)

    qT = qbAll.T @ qab  # [3072, 256] f32
    qn = [bf(qT[128 * h : 128 * (h + 1)] * inv) for h in range(H)]
    qpe = []
    for j in range(N_CORES):
        pe = bf(qT[128 * (H + j) : 128 * (H + j + 1)] * inv)
        pe2 = np.concatenate([pe[32:64], pe[0:32], pe[96:128], pe[64:96]], axis=0)
        qpe.append(bf(bf(pe * cosl) + bf(pe2 * sinl)))
    return qn, qpe, ckvn, kperB


def _sim_core2(shared, pc, qn2, qpe1, cv_g, kperB):
    """One core's stage 2 -> partial [S, D]. qn2: [2][128,S], qpe1 [128,S]."""
    bf = lambda x: x.astype(BF16).astype(np.float32)
    kbT = _untile(pc["kbT"], CV_T).astype(np.float32)
    vbT = _untile(pc["vbT"], CV_T).astype(np.float32)
    owT = _untile(pc["owT"], HPC).astype(np.float32)

    out = np.zeros((S, D), dtype=np.float32)
    for j in range(HPC):
        knT = bf(kbT[:, 128 * j : 128 * (j + 1)].T @ cv_g)
        v = bf(cv_g.T @ vbT[:, 128 * j : 128 * (j + 1)])
        qp = qpe1[64 * j : 64 * (j + 1)]
        kp = kperB[64 * j : 64 * (j + 1)]
        scores = knT.T @ qn2[j] + kp.T @ qp
        kidx = np.arange(S)[:, None]
        qidx = np.arange(S)[None, :]
        p = np.exp(scores) * (kidx <= qidx)
        p = bf(p)
        rs = p.sum(axis=0)
        oT = v.T @ p
        oT = bf(oT * (1.0 / rs))
        out += bf(oT.T @ owT[128 * j : 128 * (j + 1)])
    return out


def sim(inputs):
    shared, per_core = _prep_inputs(inputs)
    qn_all = np.zeros((H, 128, S), dtype=np.float32)
    qpe_all = np.zeros((N_CORES, 128, S), dtype=np.float32)
    cv_g = np.zeros((KV_LORA, S), dtype=np.float32)
    kperB = np.zeros((128, S), dtype=np.float32)
    for c in range(N_CORES):
        qn, qpe, cv, kp = _sim_stage1(shared, per_core[c])
        cs = slice(LOC * c, LOC * (c + 1))
        for h in range(H):
            qn_all[h][:, cs] = qn[h]
        for j in range(N_CORES):
            qpe_all[j][:, cs] = qpe[j]
        cv_g[:, cs], kperB[:, cs] = cv, kp
    out = np.zeros((S, D), dtype=np.float32)
    for c in range(N_CORES):
        out += _sim_core2(
            shared, per_core[c],
            [qn_all[2 * c], qn_all[2 * c + 1]], qpe_all[c], cv_g, kperB,
        )
    return out.reshape(B, S, D)


# ----------------------------------------------------------------------------
# bass program
# ----------------------------------------------------------------------------

def _split_waits(nc, max_waits=1):
    """This walrus build accepts at most one sem wait per instruction; hoist
    excess waits onto pure-wait EventSemaphore carriers just before it."""
    n_new = 0
    for f in nc.m.functions:
        for blk in f.blocks:
            new_insts = []
            for inst in blk.instructions:
                si = getattr(inst, "sync_info", None)
                waits = list(si.on_wait) if (si is not None and si.on_wait) else []
                if len(waits) > max_waits:
                    extra, keep = waits[:-max_waits], waits[-max_waits:]
                    for w in extra:
                        n_new += 1
                        carrier = mybir.InstEventSemaphore(
                            name=f"ws-{n_new}-{inst.name}",
                            engine=inst.engine,
                            ins=[],
                            outs=[],
                            sync_info=mybir.SyncInfo(on_wait=[w], on_update=[]),
                        )
                        nc.register_instruction(carrier, overwrite=True)
                        new_insts.append(carrier)
                    si.on_wait = keep
                new_insts.append(inst)
            blk.instructions = new_insts
    return n_new


def _build_nc():
    nc = bass.Bass(num_devices=N_CORES)
    hTloc = nc.dram_tensor("hTloc", [128, D_T * LOC], BF, kind="ExternalInput")
    qaT = nc.dram_tensor("qaT", [128, D_T * Q_LORA], BF, kind="ExternalInput")
    kvaT = nc.dram_tensor("kvaT", [128, D_T * 640], BF, kind="ExternalInput")
    qbAllT = nc.dram_tensor("qbAllT", [128, QL_T * 128 * QB_T], BF, kind="ExternalInput")
    kbT = nc.dram_tensor("kbT", [128, CV_T * 256], BF, kind="ExternalInput")
    vbT = nc.dram_tensor("vbT", [128, CV_T * 256], BF, kind="ExternalInput")
    owT = nc.dram_tensor("owT", [128, HPC * D], BF, kind="ExternalInput")
    cosl = nc.dram_tensor("cosl", [128, LOC], BF, kind="ExternalInput")
    sinl = nc.dram_tensor("sinl", [128, LOC], BF, kind="ExternalInput")
    maskb = nc.dram_tensor("maskb", [128, 1024], BF, kind="ExternalInput")
    out = nc.dram_tensor("out", [S, D], BF, kind="ExternalOutput")

    with tile.TileContext(nc) as tc:
        with tc.tile_pool(name="persist", bufs=1) as persist:
            ones_t = persist.tile([128, 128], BF, tag="ones")
            eps_t = persist.tile([128, 1], F32, tag="eps")
            nc.vector.memset(eps_t, EPS)
            nc.vector.memset(ones_t, 1.0)
            # gathered kv: [ckv 0..3 | kper 4]
            # source-major layout: gather-in destinations are contiguous
            # per partition (128 descriptors/DMA instead of 640 on SWDGE)
            cvk_g = persist.tile([128, N_CORES, CV_T + 1, LOC], BF, tag="cvkg")

            wc_cm = tc.tile_pool(name="wc", bufs=1)
            wc = wc_cm.__enter__()

            with tc.tile_pool(name="dram", bufs=1, space="DRAM") as dram:
                b_in_kv = dram.tile([128, (CV_T + 1) * LOC], BF, tag="binkv")
                b_out_kv = dram.tile(
                    [N_CORES, 128, (CV_T + 1) * LOC], BF, tag="boutkv",
                    addr_space="Shared",
                )
                b_in_q = dram.tile([N_CORES, 128, 3 * LOC], BF, tag="binq")
                b_out_q = dram.tile([N_CORES, 128, 3 * LOC], BF, tag="boutq")
                bar_d = dram.tile([1, 128], F32, tag="bard")

                # tiny barrier collective up front: absorbs cross-core start
                # skew concurrently with stage-1 compute, so the real
                # collectives later find peers already aligned
                with tc.tile_pool(name="barp", bufs=1) as barp:
                    bar_s = barp.tile([1, 128], F32, tag="bars")
                    nc.vector.memset(bar_s, 1.0)
                    nc.gpsimd.dma_start(out=bar_d[:, :], in_=bar_s)
                    nc.gpsimd.collective_compute(
                        "AllReduce",
                        mybir.AluOpType.add,
                        replica_groups=[list(range(N_CORES))],
                        ins=[bar_d[:, :].opt()],
                        outs=[bar_d[:, :].opt()],
                    )

                # ---------------- phase A: local stage 1 + q_b ----------------
                with tc.tile_pool(name="hx", bufs=1) as hx, \
                     tc.tile_pool(name="qaw", bufs=1) as qaw, \
                     tc.tile_pool(name="csl", bufs=1) as csl, \
                     tc.tile_pool(name="cvt", bufs=1) as cvt, \
                     tc.tile_pool(name="qat", bufs=1) as qat, \
                     tc.tile_pool(name="send", bufs=1) as sendp, \
                     tc.tile_pool(name="pe", bufs=1) as pep, \
                     tc.tile_pool(name="sq", bufs=1) as sqp, \
                     tc.tile_pool(name="nrm", bufs=1) as nrm, \
                     tc.tile_pool(name="acc_ps", bufs=1, space="PSUM") as acc_ps, \
                     tc.tile_pool(name="ssq_ps", bufs=1, space="PSUM") as ssq_ps, \
                     tc.tile_pool(name="ssc_ps", bufs=1, space="PSUM") as ssc_ps:

                    h_t = hx.tile([128, D_T, LOC], BF, tag="h")
                    qa_w = qaw.tile([128, D_T, Q_LORA], BF, tag="qaw")
                    cosl_t = csl.tile([128, LOC], BF, tag="cosl")
                    sinl_t = csl.tile([128, LOC], BF, tag="sinl")
                    kvw_cm = tc.tile_pool(name="kvw", bufs=1)
                    kvw = kvw_cm.__enter__()
                    kva_w = kvw.tile([128, D_T, 640], BF, tag="kvw")
                    qbw_cm = tc.tile_pool(name="qbw", bufs=1)
                    qbw = qbw_cm.__enter__()
                    qb_w = qbw.tile([128, QL_T, 128 * QB_T], BF, tag="qbw")
                    # phase-A critical DMAs first (big single transfers --
                    # fine-grained chunking measurably lowers stream bandwidth)
                    nc.sync.dma_start(
                        out=kva_w[:, 0:8, :], in_=kvaT[:, 0 : 640 * 8]
                    )
                    nc.sync.dma_start(out=h_t[:, 0:8, :], in_=hTloc[:, 0 : LOC * 8])
                    nc.sync.dma_start(
                        out=kva_w[:, 8:16, :], in_=kvaT[:, 640 * 8 : 640 * 16]
                    )
                    nc.sync.dma_start(out=h_t[:, 8:16, :], in_=hTloc[:, LOC * 8 : LOC * 16])
                    nc.sync.dma_start(out=qa_w, in_=qaT[:, :])
                    nc.sync.dma_start(out=cosl_t, in_=cosl[:, :])
                    nc.sync.dma_start(out=sinl_t, in_=sinl[:, :])
                    nc.sync.dma_start(out=qb_w, in_=qbAllT[:, :])

                    # ---- kv_a: k-outer so PE starts on the first k-chunk ----
                    cv_t = cvt.tile([128, CV_T, LOC], BF, tag="cv")
                    kvstg = cvt.tile([128, CV_T + 1, LOC], BF, tag="kvstg")
                    ssc = ssc_ps.tile([128, LOC], F32, tag="ssc")
                    kv_acc = [
                        acc_ps.tile([128, LOC], F32, tag=f"acc{m}", name=f"acc{m}")
                        for m in range(6)
                    ]
                    kcols = [
                        slice(128 * m, 128 * m + 128) if m < 4 else
                        slice(512 + 64 * (m - 4), 512 + 64 * (m - 3))
                        for m in range(6)
                    ]
                    for k in range(D_T):
                        for m in range(6):
                            nc.tensor.matmul(
                                kv_acc[m] if m < 4 else kv_acc[m][0:64, :],
                                kva_w[:, k, kcols[m]],
                                h_t[:, k, :],
                                start=(k == 0),
                                stop=(k == D_T - 1),
                            )
                    kpe_ps = [kv_acc[4][0:64, :], kv_acc[5][0:64, :]]
                    for m in range(CV_T):
                        nc.vector.tensor_copy(cv_t[:, m, :], kv_acc[m])
                        sq = sqp.tile([128, LOC], BF, tag="sq")
                        nc.scalar.activation(out=sq, in_=kv_acc[m], func=AF.Square)
                        nc.tensor.matmul(
                            ssc, ones_t, sq, start=(m == 0), stop=(m == CV_T - 1)
                        )

                    # ---- kv norm + kpe rope -> staging -> bounce -> gather ----
                    bc2 = nrm.tile([128, LOC], F32, tag="bc2")
                    nc.scalar.activation(
                        out=bc2, in_=ssc, func=AF.Sqrt, scale=1.0 / KV_LORA, bias=eps_t
                    )
                    nc.vector.reciprocal(bc2, bc2)
                    for i in range(CV_T):
                        nc.vector.tensor_mul(kvstg[:, i, :], cv_t[:, i, :], bc2)
                    t1 = nrm.tile([64, LOC], F32, tag="t1")
                    t2 = nrm.tile([64, LOC], F32, tag="t2")
                    nc.vector.tensor_mul(t1, kpe_ps[0], cosl_t[0:64, :])
                    nc.vector.tensor_mul(t2, kpe_ps[1], sinl_t[0:64, :])
                    nc.vector.tensor_add(kvstg[0:64, CV_T, :], t1, t2)
                    nc.scalar.dma_start(
                        out=kvstg[64:128, CV_T, :], in_=kvstg[0:64, CV_T, :]
                    )
                    nc.scalar.dma_start(out=b_in_kv[:, :], in_=kvstg)
                    nc.gpsimd.collective_compute(
                        "AllGather",
                        mybir.AluOpType.bypass,
                        replica_groups=[list(range(N_CORES))],
                        ins=[b_in_kv[:, :].opt()],
                        outs=[b_out_kv[:, :, :].opt()],
                    )
                    for c8 in range(N_CORES):
                        nc.gpsimd.dma_start(
                            out=cvk_g[:, c8, :, :], in_=b_out_kv[c8, :, :]
                        )

                    # ---- q_a: k-outer, two halves of 6 m-tiles ----
                    qa_t = qat.tile([128, QL_T, LOC], BF, tag="qa")
                    ssq = ssq_ps.tile([128, LOC], F32, tag="ssq")
                    for half in range(2):
                        accs = [
                            acc_ps.tile([128, LOC], F32, tag=f"acc{i}", name=f"acc{i}")
                            for i in range(6)
                        ]
                        for k in range(D_T):
                            for i in range(6):
                                m = 6 * half + i
                                nc.tensor.matmul(
                                    accs[i],
                                    qa_w[:, k, 128 * m : 128 * (m + 1)],
                                    h_t[:, k, :],
                                    start=(k == 0),
                                    stop=(k == D_T - 1),
                                )
                        for i in range(6):
                            m = 6 * half + i
                            nc.vector.tensor_copy(qa_t[:, m, :], accs[i])
                            sq = sqp.tile([128, LOC], BF, tag="sq")
                            nc.scalar.activation(out=sq, in_=accs[i], func=AF.Square)
                            nc.tensor.matmul(
                                ssq, ones_t, sq, start=(m == 0), stop=(m == QL_T - 1)
                            )
                    bc = nrm.tile([128, LOC], F32, tag="bc")
                    nc.scalar.activation(
                        out=bc, in_=ssq, func=AF.Sqrt, scale=1.0 / Q_LORA, bias=eps_t
                    )
                    nc.vector.reciprocal(bc, bc)

                    # ---- q_b all heads (24 M-tiles) ----
                    send_t = sendp.tile([128, N_CORES, 3, LOC], BF, tag="send")
                    pe_all = pep.tile([128, N_CORES, LOC], BF, tag="peall")
                    pe2_all = pep.tile([128, N_CORES, LOC], BF, tag="pe2all")
                    # pe-pair tiles FIRST so rope+swap overlaps the qn matmuls
                    for bi, b in enumerate(list(range(H, QB_T)) + list(range(H))):
                        ps = acc_ps.tile([128, LOC], F32, tag=f"acc{bi % 6}", name=f"qbps{bi % 6}")
                        for m in range(QL_T):
                            nc.tensor.matmul(
                                ps,
                                qb_w[:, m, 128 * b : 128 * (b + 1)],
                                qa_t[:, m, :],
                                start=(m == 0),
                                stop=(m == QL_T - 1),
                            )
                        if b < H:
                            nc.vector.tensor_mul(
                                send_t[:, b // 2, b % 2, :], ps, bc
                            )
                        else:
                            nc.vector.tensor_mul(pe_all[:, b - H, :], ps, bc)
                        if b == QB_T - 1:
                            # rotate-half partner via 32-row partition-swap DMAs
                            nc.sync.dma_start(out=pe2_all[0:32, :, :], in_=pe_all[32:64, :, :])
                            nc.sync.dma_start(out=pe2_all[32:64, :, :], in_=pe_all[0:32, :, :])
                            nc.sync.dma_start(out=pe2_all[64:96, :, :], in_=pe_all[96:128, :, :])
                            nc.sync.dma_start(out=pe2_all[96:128, :, :], in_=pe_all[64:96, :, :])
                            for j in range(N_CORES):
                                t1q = nrm.tile([128, LOC], F32, tag="t1q")
                                t2q = nrm.tile([128, LOC], F32, tag="t2q")
                                nc.vector.tensor_mul(t1q, pe_all[:, j, :], cosl_t)
                                nc.vector.tensor_mul(t2q, pe2_all[:, j, :], sinl_t)
                                nc.vector.tensor_add(send_t[:, j, 2, :], t1q, t2q)
                    for j in range(N_CORES):
                        nc.scalar.dma_start(out=b_in_q[j, :, :], in_=send_t[:, j, :, :])
                    qbw_cm.__exit__(None, None, None)
                    kvw_cm.__exit__(None, None, None)
                nc.gpsimd.collective_compute(
                    "AllToAll",
                    mybir.AluOpType.bypass,
                    replica_groups=[list(range(N_CORES))],
                    ins=[b_in_q[:, :, :].opt()],
                    outs=[b_out_q[:, :, :].opt()],
                )

                # ---------------- phase C: stage 2 ----------------
                with tc.tile_pool(name="qrec", bufs=1) as qrecp, \
                     tc.tile_pool(name="att", bufs=1) as att:
                    # received q: [qn(h0) | qn(h1) | qpe-pair]
                    qrec = qrecp.tile([128, 3, S], BF, tag="qrec")
                    ow_t = att.tile([128, HPC, D], BF, tag="oww")
                    mask_s = att.tile([128, 1024], BF, tag="mask")
                    kb_w = att.tile([128, CV_T, 256], BF, tag="kbw")
                    vb_w = att.tile([128, CV_T, 256], BF, tag="vbw")
                    nc.sync.dma_start(out=ow_t, in_=owT[:, :])
                    nc.sync.dma_start(out=mask_s, in_=maskb[:, :])
                    nc.sync.dma_start(out=kb_w, in_=kbT[:, :])
                    nc.sync.dma_start(out=vb_w, in_=vbT[:, :])
                    for c8 in range(N_CORES):
                        cs = slice(LOC * c8, LOC * (c8 + 1))
                        nc.gpsimd.dma_start(out=qrec[:, :, cs], in_=b_out_q[c8, :, :])

                    kn_T = [att.tile([128, S], BF, tag=f"knT{h}", name=f"knT{h}") for h in range(HPC)]
                    v_sb = [att.tile([128, S], BF, tag=f"v{h}", name=f"v{h}") for h in range(HPC)]
                    o_T = [att.tile([128, S], BF, tag=f"oT{h}", name=f"oT{h}") for h in range(HPC)]

                    # ---- kv_b: k_nope + v ----
                    with tc.tile_pool(name="kn_ps", bufs=2, space="PSUM") as kn_ps, \
                         tc.tile_pool(name="v_ps", bufs=3, space="PSUM") as v_ps:
                        for h in range(HPC):
                            hs = slice(128 * h, 128 * (h + 1))
                            for n in range(NCHUNK):
                                cs = slice(NQ * n, NQ * (n + 1))
                                ps = kn_ps.tile([128, NQ], F32, tag="knps")
                                for ct in range(CV_T):
                                    nc.tensor.matmul(
                                        ps,
                                        kb_w[:, ct, hs],
                                        cvk_g[:, 2 * n : 2 * n + 2, ct, :],
                                        start=(ct == 0),
                                        stop=(ct == CV_T - 1),
                                    )
                                nc.vector.tensor_copy(kn_T[h][:, cs], ps)
                            for kt in range(KT):
                                ks = slice(128 * kt, 128 * (kt + 1))
                                ps = v_ps.tile([128, VD], F32, tag="vps")
                                ko = 128 * (kt % 2)
                                for ct in range(CV_T):
                                    nc.tensor.matmul(
                                        ps,
                                        cvk_g[:, kt // 2, ct, ko : ko + 128],
                                        vb_w[:, ct, hs],
                                        start=(ct == 0),
                                        stop=(ct == CV_T - 1),
                                    )
                                nc.vector.tensor_copy(v_sb[h][:, ks], ps)

                    # ---------------- attention + o_proj (deferred 1 chunk) ----
                    def oproj(pool, ps_pool, si):
                        ss = slice(128 * si, 128 * (si + 1))
                        so = pool.tile([128, D], BF, tag="ostg")
                        for nch in range(NCHUNK):
                            ns = slice(NQ * nch, NQ * (nch + 1))
                            ps = ps_pool.tile([128, NQ], F32, tag="outps")
                            for j in range(HPC):
                                nc.tensor.matmul(
                                    ps,
                                    o_T[j][:, ss],
                                    ow_t[:, j, ns],
                                    start=(j == 0),
                                    stop=(j == HPC - 1),
                                )
                            nc.scalar.activation(out=so[:, ns], in_=ps, func=AF.Copy)
                        nc.sync.dma_start(out=out[ss, :], in_=so)

                    with tc.tile_pool(name="pp", bufs=6) as pp, \
                         tc.tile_pool(name="psa", bufs=2) as psa, \
                         tc.tile_pool(name="ep", bufs=3) as ep, \
                         tc.tile_pool(name="rvp", bufs=2) as rvp, \
                         tc.tile_pool(name="ostg", bufs=2) as ostg, \
                         tc.tile_pool(name="s_ps", bufs=3, space="PSUM") as s_ps, \
                         tc.tile_pool(name="rs_ps", bufs=2, space="PSUM") as rs_ps, \
                         tc.tile_pool(name="o_ps", bufs=2, space="PSUM") as o_ps, \
                         tc.tile_pool(name="out_ps", bufs=1, space="PSUM") as out_ps:
                        for c in range(NCHUNK):
                            cs = slice(NQ * c, NQ * (c + 1))
                            nkt = 4 * (c + 1)
                            for h in range(HPC):
                                rope = slice(64 * h, 64 * (h + 1))
                                rs = rs_ps.tile([128, NQ], F32, tag="rs")
                                op = o_ps.tile([128, NQ], F32, tag="op")
                                psum_v = psa.tile([128, NQ], F32, tag="psumv")
                                for kt in range(nkt):
                                    ks = slice(128 * kt, 128 * (kt + 1))
                                    i = kt - 4 * c
                                    lo = 128 * i if i > 0 else 0
                                    qs = slice(NQ * c + lo, NQ * (c + 1))
                                    vs = slice(lo, NQ)
                                    sp = s_ps.tile([128, NQ], F32, tag="sp")
                                    nc.tensor.matmul(
                                        sp[:, vs], kn_T[h][:, ks], qrec[:, h, qs],
                                        start=True, stop=False,
                                    )
                                    nc.tensor.matmul(
                                        sp[:, vs],
                                        cvk_g[rope, kt // 2, CV_T,
                                              128 * (kt % 2) : 128 * (kt % 2) + 128],
                                        qrec[rope, 2, qs],
                                        start=False, stop=True,
                                    )
                                    p_t = pp.tile([128, NQ], BF, tag="p")
                                    if kt >= 4 * c:
                                        e_t = ep.tile([128, NQ], BF, tag="e")
                                        nc.scalar.activation(out=e_t[:, vs], in_=sp[:, vs], func=AF.Exp)
                                        nc.vector.tensor_mul(
                                            p_t[:, vs], e_t[:, vs],
                                            mask_s[:, 384 : 896 - lo],
                                        )
                                    else:
                                        nc.scalar.activation(out=p_t[:, vs], in_=sp[:, vs], func=AF.Exp)
                                    # accumulate p on vector (k-partial rowsums)
                                    if kt == 0:
                                        nc.vector.tensor_copy(psum_v, p_t)
                                    else:
                                        nc.vector.tensor_add(
                                            psum_v[:, vs], psum_v[:, vs], p_t[:, vs]
                                        )
                                    nc.tensor.matmul(
                                        op[:, vs],
                                        v_sb[h][:, ks],
                                        p_t[:, vs],
                                        start=(kt == 0), stop=(kt == nkt - 1),
                                    )
                                pcast = psa.tile([128, NQ], BF, tag="pcast")
                                nc.vector.tensor_copy(pcast, psum_v)
                                nc.tensor.matmul(rs, ones_t, pcast, start=True, stop=True)
                                rv = rvp.tile([128, NQ], F32, tag="rv")
                                nc.vector.reciprocal(rv, rs)
                                nc.vector.tensor_mul(o_T[h][:, cs], op, rv)
                            # o_proj for the PREVIOUS chunk (o_T long ready)
                            if c >= 1:
                                for si in range(4 * (c - 1), 4 * c):
                                    oproj(ostg, out_ps, si)
                    # ---------------- final chunk o_proj ----------------
                    with tc.tile_pool(name="ostg2", bufs=4) as ostg2, \
                         tc.tile_pool(name="out2_ps", bufs=4, space="PSUM") as out2_ps:
                        for si in range(4 * (NCHUNK - 1), 4 * NCHUNK):
                            oproj(ostg2, out2_ps, si)
            wc_cm.__exit__(None, None, None)
    _split_waits(nc)
    return nc


# ----------------------------------------------------------------------------
# entry point
# ----------------------------------------------------------------------------

def kernel(**inputs):
    global LAST_RESULTS
    shared, per_core = _prep_inputs(inputs)
    if "nc" not in _CACHE:
        _CACHE["nc"] = _build_nc()
    nc = _CACHE["nc"]
    in_maps = []
    for c in range(N_CORES):
        m = {
            "qaT": shared["qaT"],
            "kvaT": shared["kvaT"],
            "qbAllT": shared["qbAllT"],
            "maskb": shared["maskb"],
            "hTloc": per_core[c]["hTloc"],
            "cosl": per_core[c]["cosl"],
            "sinl": per_core[c]["sinl"],
            "kbT": per_core[c]["kbT"],
            "vbT": per_core[c]["vbT"],
            "owT": per_core[c]["owT"],
        }
        in_maps.append(m)
    res = run_bass_kernel_spmd(nc, in_maps, core_ids=list(range(N_CORES)))
    LAST_RESULTS = res
    out = np.zeros((S, D), dtype=np.float32)
    for r in res.results:
        out += np.asarray(r["out"], dtype=np.float32)
    return out.reshape(B, S, D)


# revision 24
# speedup vs baseline: 1.0455x; 1.0455x over previous
"""DeepseekV3 MLA attention (B=1, S=2048, D=2048, H=16) on 8 trn2 NeuronCores.

v3 strategy:
  - stage 1 (q_a / kv_a + rmsnorm + rope) AND q_b (for ALL 16 heads) are
    SEQUENCE-SHARDED: core c computes them only for its 256-token chunk;
  - kv activations (ckvn 4 tiles + kper 1 tile, [128,256] bf16) are
    exchanged with an early HBM AllGather (0.33 MB in -> 2.6 MB out) that
    overlaps the q_a/q_b compute;
  - q heads are exchanged with an AllToAll (1.57 MB): core c sends, for each
    destination j, [qn(2j) | qn(2j+1) | qpe-pair(j)] on its local tokens;
  - stage 2 (kv_b, causal flash attention, o_proj slice for 2 owned heads)
    is tensor-parallel over heads; host sums bf16 partials.

RoPE: deinterleave folded into weights; rotate-half partner produced by a
32-row partition-swap DMA with the sign folded into the sin table. rmsnorm
inv scale folded into the PSUM->SBUF copies after q_b (per-token scalar
commutes through the linear map).

All weights shipped in partition-major tiled layout [128, ktiles*cols] so
each SBUF weight load is one DMA.
"""

import numpy as np
import ml_dtypes

import concourse.bass as bass
import concourse.mybir as mybir
import concourse.tile as tile
from concourse.bass_utils import run_bass_kernel_spmd

BF16 = ml_dtypes.bfloat16
F32 = mybir.dt.float32
BF = mybir.dt.bfloat16

B, S, D = 1, 2048, 2048
H = 16
N_CORES = 8
HPC = H // N_CORES  # heads per core = 2
Q_LORA = 1536
KV_LORA = 512
NOPE = 128
ROPE = 64
VD = 128
QHD = NOPE + ROPE  # 192
THETA = 50000.0
EPS = 1e-6
SCALE = QHD ** (-0.5)

LOC = S // N_CORES   # 256 local chunk
NQ = 512             # q-chunk (matmul free dim) in stage 2
NCHUNK = S // NQ     # 4
KT = S // 128        # 16 k-tiles
QL_T = Q_LORA // 128  # 12
D_T = D // 128        # 16
CV_T = KV_LORA // 128  # 4
QB_T = H + N_CORES    # 24 q_b output tiles: 16 nope + 8 pe-pairs
AF = mybir.ActivationFunctionType

LAST_RESULTS = None
_CACHE = {}


def _tiled(a, rows=128):
    """[kt*rows, cols] -> partition-major [rows, kt*cols] (single-DMA load)."""
    kt = a.shape[0] // rows
    return np.ascontiguousarray(
        a.reshape(kt, rows, a.shape[1]).transpose(1, 0, 2).reshape(rows, -1)
    )


# ----------------------------------------------------------------------------
# host-side weight preparation
# ----------------------------------------------------------------------------

def _deint_perm():
    p = np.empty(ROPE, dtype=np.int64)
    p[:32] = 2 * np.arange(32)
    p[32:] = 2 * np.arange(32) + 1
    return p


def _rope_tables(position_ids):
    pos = np.asarray(position_ids).reshape(-1).astype(np.float32)  # [S]
    inv_freq = (1.0 / (THETA ** (np.arange(0, ROPE, 2, dtype=np.float32) / ROPE)))
    freqs = np.outer(pos, inv_freq)  # [S, 32]
    cos32 = np.cos(freqs).T.astype(np.float32)  # [32, S]
    sin32 = np.sin(freqs).T.astype(np.float32)
    cos128 = np.tile(cos32, (4, 1))  # [128, S]
    sin128 = np.tile(sin32, (4, 1))
    # rotate-half sign folded into sin: row j multiplies the swapped partner,
    # with sign -1 for j%64 < 32
    sgn = np.where((np.arange(128) % 64) < 32, -1.0, 1.0).astype(np.float32)
    sin128s = sgn[:, None] * sin128
    return cos128, sin128s


def _causal_mask_big():
    dk = np.arange(128)[:, None]
    u = np.arange(1024)[None, :]
    return (u >= dk + 384).astype(BF16)


def _prep_inputs(inputs):
    hidden = np.asarray(inputs["hidden_states"], dtype=np.float32)[0]  # [S, D]
    position_ids = np.asarray(inputs["position_ids"])
    q_a_w = np.asarray(inputs["q_a_w"], dtype=np.float32)
    q_a_ln_w = np.asarray(inputs["q_a_ln_w"], dtype=np.float32)
    q_b_w = np.asarray(inputs["q_b_w"], dtype=np.float32)
    kv_a_w = np.asarray(inputs["kv_a_w"], dtype=np.float32)
    kv_a_ln_w = np.asarray(inputs["kv_a_ln_w"], dtype=np.float32)
    kv_b_w = np.asarray(inputs["kv_b_w"], dtype=np.float32)
    o_w = np.asarray(inputs["o_w"], dtype=np.float32)

    dp = _deint_perm()
    dps = dp[(np.arange(ROPE) ^ 32)]

    hT = np.ascontiguousarray(hidden.T).astype(BF16)  # [D, S]

    shared = {}
    shared["qaT"] = _tiled(np.ascontiguousarray(q_a_w.T).astype(BF16))
    kva_cols = np.concatenate(
        [kv_a_w[:KV_LORA], kv_a_w[KV_LORA + dp], kv_a_w[KV_LORA + dps]], axis=0
    )  # [640, D]
    shared["kvaT"] = _tiled(np.ascontiguousarray(kva_cols.T).astype(BF16))

    # q_b for ALL heads: [16 nope tiles | 8 pe-pair tiles] x 1536
    qb = (q_b_w * q_a_ln_w[None, :] * SCALE).reshape(H, QHD, Q_LORA)
    rows = [qb[h, :NOPE] for h in range(H)]
    for j in range(N_CORES):
        rows.append(
            np.concatenate([qb[2 * j, NOPE + dp], qb[2 * j + 1, NOPE + dp]], axis=0)
        )
    qball = np.concatenate(rows, axis=0)  # [24*128, 1536]
    shared["qbAllT"] = _tiled(np.ascontiguousarray(qball.T).astype(BF16))  # [128, 12*3072]

    cos128, sin128s = _rope_tables(position_ids)
    shared["maskb"] = _causal_mask_big()

    kvb = (kv_b_w * kv_a_ln_w[None, :]).reshape(H, NOPE + VD, KV_LORA)

    per_core = []
    for c in range(N_CORES):
        h0, h1 = HPC * c, HPC * c + 1
        kb_cols = np.concatenate([kvb[h0, :NOPE], kvb[h1, :NOPE]], axis=0)
        vb_cols = np.concatenate([kvb[h0, NOPE:], kvb[h1, NOPE:]], axis=0)
        o_slice = o_w[:, VD * h0 : VD * (h1 + 1)]
        cl = cos128[:, LOC * c : LOC * (c + 1)]  # [128, 256]
        sl = sin128s[:, LOC * c : LOC * (c + 1)]
        per_core.append(
            {
                "hTloc": _tiled(np.ascontiguousarray(hT[:, LOC * c : LOC * (c + 1)])),
                "cosl": np.ascontiguousarray(cl).astype(BF16),  # [128, 256]
                "sinl": np.ascontiguousarray(sl).astype(BF16),
                "kbT": _tiled(np.ascontiguousarray(kb_cols.T).astype(BF16)),
                "vbT": _tiled(np.ascontiguousarray(vb_cols.T).astype(BF16)),
                "owT": _tiled(np.ascontiguousarray(o_slice.T).astype(BF16)),
            }
        )
    return shared, per_core


# ----------------------------------------------------------------------------
# numpy simulation of the device program (for host-side validation)
# ----------------------------------------------------------------------------

def _untile(a, kt):
    return a.reshape(128, kt, -1).transpose(1, 0, 2).reshape(128 * kt, -1)


def _sim_stage1(shared, pc):
    """One core's stage 1+q_b on its local chunk.

    Returns (qn [16][128,256], qpe [8 pairs][128,256], ckvn, kperB) bf16."""
    bf = lambda x: x.astype(BF16).astype(np.float32)
    hT = _untile(pc["hTloc"], D_T).astype(np.float32)
    qaT = _untile(shared["qaT"], D_T).astype(np.float32)
    kvaT = _untile(shared["kvaT"], D_T).astype(np.float32)
    qbAll = _untile(shared["qbAllT"], QL_T).astype(np.float32)  # [1536, 3072]
    cosl = pc["cosl"].astype(np.float32)
    sinl = pc["sinl"].astype(np.float32)

    ckvT = kvaT.T @ hT
    ckv = ckvT[:KV_LORA]
    ckvb = bf(ckv)
    ssc = (bf(ckvb * ckvb)).sum(axis=0)
    invc = 1.0 / np.sqrt(ssc / KV_LORA + EPS)
    ckvn = bf(ckvb * invc)
    kpe, kpe2 = ckvT[512:576], ckvT[576:640]
    kper = bf(kpe * cosl[0:64] + kpe2 * sinl[0:64])
    kperB = np.concatenate([kper, kper], axis=0)

    qaTx = qaT.T @ hT
    qab = bf(qaTx)
    ssq = (bf(qab * qab)).sum(axis=0)
    inv = 1.0 / np.sqrt(ssq / Q_LORA + EPS)

    qT = qbAll.T @ qab  # [3072, 256] f32
    qn = [bf(qT[128 * h : 128 * (h + 1)] * inv) for h in range(H)]
    qpe = []
    for j in range(N_CORES):
        pe = bf(qT[128 * (H + j) : 128 * (H + j + 1)] * inv)
        pe2 = np.concatenate([pe[32:64], pe[0:32], pe[96:128], pe[64:96]], axis=0)
        qpe.append(bf(bf(pe * cosl) + bf(pe2 * sinl)))
    return qn, qpe, ckvn, kperB


def _sim_core2(shared, pc, qn2, qpe1, cv_g, kperB):
    """One core's stage 2 -> partial [S, D]. qn2: [2][128,S], qpe1 [128,S]."""
    bf = lambda x: x.astype(BF16).astype(np.float32)
    kbT = _untile(pc["kbT"], CV_T).astype(np.float32)
    vbT = _untile(pc["vbT"], CV_T).astype(np.float32)
    owT = _untile(pc["owT"], HPC).astype(np.float32)

    out = np.zeros((S, D), dtype=np.float32)
    for j in range(HPC):
        knT = bf(kbT[:, 128 * j : 128 * (j + 1)].T @ cv_g)
        v = bf(cv_g.T @ vbT[:, 128 * j : 128 * (j + 1)])
        qp = qpe1[64 * j : 64 * (j + 1)]
        kp = kperB[64 * j : 64 * (j + 1)]
        scores = knT.T @ qn2[j] + kp.T @ qp
        kidx = np.arange(S)[:, None]
        qidx = np.arange(S)[None, :]
        p = np.exp(scores) * (kidx <= qidx)
        p = bf(p)
        rs = p.sum(axis=0)
        oT = v.T @ p
        oT = bf(oT * (1.0 / rs))
        out += bf(oT.T @ owT[128 * j : 128 * (j + 1)])
    return out


def sim(inputs):
    shared, per_core = _prep_inputs(inputs)
    qn_all = np.zeros((H, 128, S), dtype=np.float32)
    qpe_all = np.zeros((N_CORES, 128, S), dtype=np.float32)
    cv_g = np.zeros((KV_LORA, S), dtype=np.float32)
    kperB = np.zeros((128, S), dtype=np.float32)
    for c in range(N_CORES):
        qn, qpe, cv, kp = _sim_stage1(shared, per_core[c])
        cs = slice(LOC * c, LOC * (c + 1))
        for h in range(H):
            qn_all[h][:, cs] = qn[h]
        for j in range(N_CORES):
            qpe_all[j][:, cs] = qpe[j]
        cv_g[:, cs], kperB[:, cs] = cv, kp
    out = np.zeros((S, D), dtype=np.float32)
    for c in range(N_CORES):
        out += _sim_core2(
            shared, per_core[c],
            [qn_all[2 * c], qn_all[2 * c + 1]], qpe_all[c], cv_g, kperB,
        )
    return out.reshape(B, S, D)


# ----------------------------------------------------------------------------
# bass program
# ----------------------------------------------------------------------------

def _split_waits(nc, max_waits=1):
    """This walrus build accepts at most one sem wait per instruction; hoist
    excess waits onto pure-wait EventSemaphore carriers just before it."""
    n_new = 0
    for f in nc.m.functions:
        for blk in f.blocks:
            new_insts = []
            for inst in blk.instructions:
                si = getattr(inst, "sync_info", None)
                waits = list(si.on_wait) if (si is not None and si.on_wait) else []
                if len(waits) > max_waits:
                    extra, keep = waits[:-max_waits], waits[-max_waits:]
                    for w in extra:
                        n_new += 1
                        carrier = mybir.InstEventSemaphore(
                            name=f"ws-{n_new}-{inst.name}",
                            engine=inst.engine,
                            ins=[],
                            outs=[],
                            sync_info=mybir.SyncInfo(on_wait=[w], on_update=[]),
                        )
                        nc.register_instruction(carrier, overwrite=True)
                        new_insts.append(carrier)
                    si.on_wait = keep
                new_insts.append(inst)
            blk.instructions = new_insts
    return n_new


def _build_nc():
    nc = bass.Bass(num_devices=N_CORES)
    hTloc = nc.dram_tensor("hTloc", [128, D_T * LOC], BF, kind="ExternalInput")
    qaT = nc.dram_tensor("qaT", [128, D_T * Q_LORA], BF, kind="ExternalInput")
    kvaT = nc.dram_tensor("kvaT", [128, D_T * 640], BF, kind="ExternalInput")
    qbAllT = nc.dram_tensor("qbAllT", [128, QL_T * 128 * QB_T], BF, kind="ExternalInput")
    kbT = nc.dram_tensor("kbT", [128, CV_T * 256], BF, kind="ExternalInput")
    vbT = nc.dram_tensor("vbT", [128, CV_T * 256], BF, kind="ExternalInput")
    owT = nc.dram_tensor("owT", [128, HPC * D], BF, kind="ExternalInput")
    cosl = nc.dram_tensor("cosl", [128, LOC], BF, kind="ExternalInput")
    sinl = nc.dram_tensor("sinl", [128, LOC], BF, kind="ExternalInput")
    maskb = nc.dram_tensor("maskb", [128, 1024], BF, kind="ExternalInput")
    out = nc.dram_tensor("out", [S, D], BF, kind="ExternalOutput")

    with tile.TileContext(nc) as tc:
        with tc.tile_pool(name="persist", bufs=1) as persist:
            ones_t = persist.tile([128, 128], BF, tag="ones")
            eps_t = persist.tile([128, 1], F32, tag="eps")
            nc.vector.memset(eps_t, EPS)
            nc.vector.memset(ones_t, 1.0)
            # gathered kv: [ckv 0..3 | kper 4]
            # source-major layout: gather-in destinations are contiguous
            # per partition (128 descriptors/DMA instead of 640 on SWDGE)
            cvk_g = persist.tile([128, N_CORES, CV_T + 1, LOC], BF, tag="cvkg")

            wc_cm = tc.tile_pool(name="wc", bufs=1)
            wc = wc_cm.__enter__()
            kb_w = wc.tile([128, CV_T, 256], BF, tag="kbw")
            vb_w = wc.tile([128, CV_T, 256], BF, tag="vbw")

            with tc.tile_pool(name="dram", bufs=1, space="DRAM") as dram:
                b_in_kv = dram.tile([128, (CV_T + 1) * LOC], BF, tag="binkv")
                b_out_kv = dram.tile(
                    [N_CORES, 128, (CV_T + 1) * LOC], BF, tag="boutkv",
                    addr_space="Shared",
                )
                b_in_q = dram.tile([N_CORES, 128, 3 * LOC], BF, tag="binq")
                b_out_q = dram.tile([N_CORES, 128, 3 * LOC], BF, tag="boutq")
                bar_d = dram.tile([1, 128], F32, tag="bard")

                # tiny barrier collective up front: absorbs cross-core start
                # skew concurrently with stage-1 compute, so the real
                # collectives later find peers already aligned
                with tc.tile_pool(name="barp", bufs=1) as barp:
                    bar_s = barp.tile([1, 128], F32, tag="bars")
                    nc.vector.memset(bar_s, 1.0)
                    nc.gpsimd.dma_start(out=bar_d[:, :], in_=bar_s)
                    nc.gpsimd.collective_compute(
                        "AllReduce",
                        mybir.AluOpType.add,
                        replica_groups=[list(range(N_CORES))],
                        ins=[bar_d[:, :].opt()],
                        outs=[bar_d[:, :].opt()],
                    )

                # ---------------- phase A: local stage 1 + q_b ----------------
                with tc.tile_pool(name="hx", bufs=1) as hx, \
                     tc.tile_pool(name="qaw", bufs=1) as qaw, \
                     tc.tile_pool(name="csl", bufs=1) as csl, \
                     tc.tile_pool(name="cvt", bufs=1) as cvt, \
                     tc.tile_pool(name="qat", bufs=1) as qat, \
                     tc.tile_pool(name="send", bufs=1) as sendp, \
                     tc.tile_pool(name="pe", bufs=1) as pep, \
                     tc.tile_pool(name="sq", bufs=2) as sqp, \
                     tc.tile_pool(name="nrm", bufs=2) as nrm, \
                     tc.tile_pool(name="acc_ps", bufs=1, space="PSUM") as acc_ps, \
                     tc.tile_pool(name="ssq_ps", bufs=1, space="PSUM") as ssq_ps, \
                     tc.tile_pool(name="ssc_ps", bufs=1, space="PSUM") as ssc_ps:

                    h_t = hx.tile([128, D_T, LOC], BF, tag="h")
                    qa_w = qaw.tile([128, D_T, Q_LORA], BF, tag="qaw")
                    cosl_t = csl.tile([128, LOC], BF, tag="cosl")
                    sinl_t = csl.tile([128, LOC], BF, tag="sinl")
                    kvw_cm = tc.tile_pool(name="kvw", bufs=1)
                    kvw = kvw_cm.__enter__()
                    kva_w = kvw.tile([128, D_T, 640], BF, tag="kvw")
                    # phase-A critical DMAs first (big single transfers --
                    # fine-grained chunking measurably lowers stream bandwidth)
                    nc.sync.dma_start(
                        out=kva_w[:, 0:8, :], in_=kvaT[:, 0 : 640 * 8]
                    )
                    nc.sync.dma_start(out=h_t[:, 0:8, :], in_=hTloc[:, 0 : LOC * 8])
                    nc.sync.dma_start(
                        out=kva_w[:, 8:16, :], in_=kvaT[:, 640 * 8 : 640 * 16]
                    )
                    nc.sync.dma_start(out=h_t[:, 8:16, :], in_=hTloc[:, LOC * 8 : LOC * 16])
                    nc.sync.dma_start(out=qa_w, in_=qaT[:, :])
                    nc.sync.dma_start(out=cosl_t, in_=cosl[:, :])
                    nc.sync.dma_start(out=sinl_t, in_=sinl[:, :])
                    nc.sync.dma_start(out=kb_w, in_=kbT[:, :])
                    nc.sync.dma_start(out=vb_w, in_=vbT[:, :])

                    # ---- kv_a: k-outer so PE starts on the first k-chunk ----
                    cv_t = cvt.tile([128, CV_T, LOC], BF, tag="cv")
                    kvstg = cvt.tile([128, CV_T + 1, LOC], BF, tag="kvstg")
                    ssc = ssc_ps.tile([128, LOC], F32, tag="ssc")
                    kv_acc = [
                        acc_ps.tile([128, LOC], F32, tag=f"acc{m}", name=f"acc{m}")
                        for m in range(6)
                    ]
                    kcols = [
                        slice(128 * m, 128 * m + 128) if m < 4 else
                        slice(512 + 64 * (m - 4), 512 + 64 * (m - 3))
                        for m in range(6)
                    ]
                    for k in range(D_T):
                        for m in range(6):
                            nc.tensor.matmul(
                                kv_acc[m] if m < 4 else kv_acc[m][0:64, :],
                                kva_w[:, k, kcols[m]],
                                h_t[:, k, :],
                                start=(k == 0),
                                stop=(k == D_T - 1),
                            )
                    kpe_ps = [kv_acc[4][0:64, :], kv_acc[5][0:64, :]]
                    for m in range(CV_T):
                        nc.vector.tensor_copy(cv_t[:, m, :], kv_acc[m])
                        sq = sqp.tile([128, LOC], BF, tag="sq")
                        nc.scalar.activation(out=sq, in_=kv_acc[m], func=AF.Square)
                        nc.tensor.matmul(
                            ssc, ones_t, sq, start=(m == 0), stop=(m == CV_T - 1)
                        )

                    # ---- kv norm + kpe rope -> staging -> bounce -> gather ----
                    bc2 = nrm.tile([128, LOC], F32, tag="bc2")
                    nc.scalar.activation(
                        out=bc2, in_=ssc, func=AF.Sqrt, scale=1.0 / KV_LORA, bias=eps_t
                    )
                    nc.vector.reciprocal(bc2, bc2)
                    for i in range(CV_T):
                        nc.vector.tensor_mul(kvstg[:, i, :], cv_t[:, i, :], bc2)
                    t1 = nrm.tile([64, LOC], F32, tag="t1")
                    t2 = nrm.tile([64, LOC], F32, tag="t2")
                    nc.vector.tensor_mul(t1, kpe_ps[0], cosl_t[0:64, :])
                    nc.vector.tensor_mul(t2, kpe_ps[1], sinl_t[0:64, :])
                    nc.vector.tensor_add(kvstg[0:64, CV_T, :], t1, t2)
                    nc.scalar.dma_start(
                        out=kvstg[64:128, CV_T, :], in_=kvstg[0:64, CV_T, :]
                    )
                    nc.scalar.dma_start(out=b_in_kv[:, :], in_=kvstg)
                    nc.gpsimd.collective_compute(
                        "AllGather",
                        mybir.AluOpType.bypass,
                        replica_groups=[list(range(N_CORES))],
                        ins=[b_in_kv[:, :].opt()],
                        outs=[b_out_kv[:, :, :].opt()],
                    )
                    for c8 in range(N_CORES):
                        nc.gpsimd.dma_start(
                            out=cvk_g[:, c8, :, :], in_=b_out_kv[c8, :, :]
                        )

                    # free kva (top of SBUF pool stack), load qbAll into its
                    # place so it streams during the q_a passes
                    kvw_cm.__exit__(None, None, None)
                    qbw_cm = tc.tile_pool(name="qbw", bufs=1)
                    qbw = qbw_cm.__enter__()
                    qb_w = qbw.tile([128, QL_T, 128 * QB_T], BF, tag="qbw")
                    nc.sync.dma_start(out=qb_w, in_=qbAllT[:, :])

                    # ---- q_a: k-outer, two halves of 6 m-tiles ----
                    qa_t = qat.tile([128, QL_T, LOC], BF, tag="qa")
                    ssq = ssq_ps.tile([128, LOC], F32, tag="ssq")
                    for half in range(2):
                        accs = [
                            acc_ps.tile([128, LOC], F32, tag=f"acc{i}", name=f"acc{i}")
                            for i in range(6)
                        ]
                        for k in range(D_T):
                            for i in range(6):
                                m = 6 * half + i
                                nc.tensor.matmul(
                                    accs[i],
                                    qa_w[:, k, 128 * m : 128 * (m + 1)],
                                    h_t[:, k, :],
                                    start=(k == 0),
                                    stop=(k == D_T - 1),
                                )
                        for i in range(6):
                            m = 6 * half + i
                            nc.vector.tensor_copy(qa_t[:, m, :], accs[i])
                            sq = sqp.tile([128, LOC], BF, tag="sq")
                            nc.scalar.activation(out=sq, in_=accs[i], func=AF.Square)
                            nc.tensor.matmul(
                                ssq, ones_t, sq, start=(m == 0), stop=(m == QL_T - 1)
                            )
                    bc = nrm.tile([128, LOC], F32, tag="bc")
                    nc.scalar.activation(
                        out=bc, in_=ssq, func=AF.Sqrt, scale=1.0 / Q_LORA, bias=eps_t
                    )
                    nc.vector.reciprocal(bc, bc)

                    # ---- q_b all heads (24 M-tiles) ----
                    send_t = sendp.tile([128, N_CORES, 3, LOC], BF, tag="send")
                    pe_all = pep.tile([128, N_CORES, LOC], BF, tag="peall")
                    pe2_all = pep.tile([128, N_CORES, LOC], BF, tag="pe2all")
                    # pe-pair tiles FIRST so rope+swap overlaps the qn matmuls
                    for bi, b in enumerate(list(range(H, QB_T)) + list(range(H))):
                        ps = acc_ps.tile([128, LOC], F32, tag=f"acc{bi % 6}", name=f"qbps{bi % 6}")
                        for m in range(QL_T):
                            nc.tensor.matmul(
                                ps,
                                qb_w[:, m, 128 * b : 128 * (b + 1)],
                                qa_t[:, m, :],
                                start=(m == 0),
                                stop=(m == QL_T - 1),
                            )
                        if b < H:
                            nc.vector.tensor_mul(
                                send_t[:, b // 2, b % 2, :], ps, bc
                            )
                        else:
                            nc.vector.tensor_mul(pe_all[:, b - H, :], ps, bc)
                        if b == QB_T - 1:
                            # rotate-half partner via 32-row partition-swap DMAs
                            nc.sync.dma_start(out=pe2_all[0:32, :, :], in_=pe_all[32:64, :, :])
                            nc.sync.dma_start(out=pe2_all[32:64, :, :], in_=pe_all[0:32, :, :])
                            nc.sync.dma_start(out=pe2_all[64:96, :, :], in_=pe_all[96:128, :, :])
                            nc.sync.dma_start(out=pe2_all[96:128, :, :], in_=pe_all[64:96, :, :])
                            for j in range(N_CORES):
                                t1q = nrm.tile([128, LOC], F32, tag="t1q")
                                t2q = nrm.tile([128, LOC], F32, tag="t2q")
                                nc.vector.tensor_mul(t1q, pe_all[:, j, :], cosl_t)
                                nc.vector.tensor_mul(t2q, pe2_all[:, j, :], sinl_t)
                                nc.vector.tensor_add(send_t[:, j, 2, :], t1q, t2q)
                    for j in range(N_CORES):
                        nc.scalar.dma_start(out=b_in_q[j, :, :], in_=send_t[:, j, :, :])
                    qbw_cm.__exit__(None, None, None)
                nc.gpsimd.collective_compute(
                    "AllToAll",
                    mybir.AluOpType.bypass,
                    replica_groups=[list(range(N_CORES))],
                    ins=[b_in_q[:, :, :].opt()],
                    outs=[b_out_q[:, :, :].opt()],
                )

                # ---------------- phase C: stage 2 ----------------
                with tc.tile_pool(name="qrec", bufs=1) as qrecp, \
                     tc.tile_pool(name="att", bufs=1) as att:
                    # received q: [qn(h0) | qn(h1) | qpe-pair]
                    qrec = qrecp.tile([128, 3, S], BF, tag="qrec")
                    ow_t = att.tile([128, HPC, D], BF, tag="oww")
                    mask_s = att.tile([128, 1024], BF, tag="mask")
                    nc.sync.dma_start(out=ow_t, in_=owT[:, :])
                    nc.sync.dma_start(out=mask_s, in_=maskb[:, :])
                    for c8 in range(N_CORES):
                        cs = slice(LOC * c8, LOC * (c8 + 1))
                        nc.gpsimd.dma_start(out=qrec[:, :, cs], in_=b_out_q[c8, :, :])

                    kn_T = [att.tile([128, S], BF, tag=f"knT{h}", name=f"knT{h}") for h in range(HPC)]
                    v_sb = [att.tile([128, S], BF, tag=f"v{h}", name=f"v{h}") for h in range(HPC)]
                    o_T = [att.tile([128, S], BF, tag=f"oT{h}", name=f"oT{h}") for h in range(HPC)]

                    # ---- kv_b: k_nope + v ----
                    with tc.tile_pool(name="kn_ps", bufs=2, space="PSUM") as kn_ps, \
                         tc.tile_pool(name="v_ps", bufs=3, space="PSUM") as v_ps:
                        for h in range(HPC):
                            hs = slice(128 * h, 128 * (h + 1))
                            for n in range(NCHUNK):
                                cs = slice(NQ * n, NQ * (n + 1))
                                ps = kn_ps.tile([128, NQ], F32, tag="knps")
                                for ct in range(CV_T):
                                    nc.tensor.matmul(
                                        ps,
                                        kb_w[:, ct, hs],
                                        cvk_g[:, 2 * n : 2 * n + 2, ct, :],
                                        start=(ct == 0),
                                        stop=(ct == CV_T - 1),
                                    )
                                nc.vector.tensor_copy(kn_T[h][:, cs], ps)
                            for kt in range(KT):
                                ks = slice(128 * kt, 128 * (kt + 1))
                                ps = v_ps.tile([128, VD], F32, tag="vps")
                                ko = 128 * (kt % 2)
                                for ct in range(CV_T):
                                    nc.tensor.matmul(
                                        ps,
                                        cvk_g[:, kt // 2, ct, ko : ko + 128],
                                        vb_w[:, ct, hs],
                                        start=(ct == 0),
                                        stop=(ct == CV_T - 1),
                                    )
                                nc.vector.tensor_copy(v_sb[h][:, ks], ps)

                    # ---------------- attention + o_proj (deferred 1 chunk) ----
                    def oproj(pool, ps_pool, si):
                        ss = slice(128 * si, 128 * (si + 1))
                        so = pool.tile([128, D], BF, tag="ostg")
                        for nch in range(NCHUNK):
                            ns = slice(NQ * nch, NQ * (nch + 1))
                            ps = ps_pool.tile([128, NQ], F32, tag="outps")
                            for j in range(HPC):
                                nc.tensor.matmul(
                                    ps,
                                    o_T[j][:, ss],
                                    ow_t[:, j, ns],
                                    start=(j == 0),
                                    stop=(j == HPC - 1),
                                )
                            nc.scalar.activation(out=so[:, ns], in_=ps, func=AF.Copy)
                        nc.sync.dma_start(out=out[ss, :], in_=so)

                    with tc.tile_pool(name="pp", bufs=6) as pp, \
                         tc.tile_pool(name="psa", bufs=2) as psa, \
                         tc.tile_pool(name="ep", bufs=3) as ep, \
                         tc.tile_pool(name="rvp", bufs=2) as rvp, \
                         tc.tile_pool(name="ostg", bufs=2) as ostg, \
                         tc.tile_pool(name="s_ps", bufs=3, space="PSUM") as s_ps, \
                         tc.tile_pool(name="rs_ps", bufs=2, space="PSUM") as rs_ps, \
                         tc.tile_pool(name="o_ps", bufs=2, space="PSUM") as o_ps, \
                         tc.tile_pool(name="out_ps", bufs=1, space="PSUM") as out_ps:
                        for c in range(NCHUNK):
                            cs = slice(NQ * c, NQ * (c + 1))
                            nkt = 4 * (c + 1)
                            for h in range(HPC):
                                rope = slice(64 * h, 64 * (h + 1))
                                rs = rs_ps.tile([128, NQ], F32, tag="rs")
                                op = o_ps.tile([128, NQ], F32, tag="op")
                                psum_v = psa.tile([128, NQ], F32, tag="psumv")
                                for kt in range(nkt):
                                    ks = slice(128 * kt, 128 * (kt + 1))
                                    i = kt - 4 * c
                                    lo = 128 * i if i > 0 else 0
                                    qs = slice(NQ * c + lo, NQ * (c + 1))
                                    vs = slice(lo, NQ)
                                    sp = s_ps.tile([128, NQ], F32, tag="sp")
                                    nc.tensor.matmul(
                                        sp[:, vs], kn_T[h][:, ks], qrec[:, h, qs],
                                        start=True, stop=False,
                                    )
                                    nc.tensor.matmul(
                                        sp[:, vs],
                                        cvk_g[rope, kt // 2, CV_T,
                                              128 * (kt % 2) : 128 * (kt % 2) + 128],
                                        qrec[rope, 2, qs],
                                        start=False, stop=True,
                                    )
                                    p_t = pp.tile([128, NQ], BF, tag="p")
                                    if kt >= 4 * c:
                                        e_t = ep.tile([128, NQ], BF, tag="e")
                                        nc.scalar.activation(out=e_t[:, vs], in_=sp[:, vs], func=AF.Exp)
                                        nc.vector.tensor_mul(
                                            p_t[:, vs], e_t[:, vs],
                                            mask_s[:, 384 : 896 - lo],
                                        )
                                    else:
                                        nc.scalar.activation(out=p_t[:, vs], in_=sp[:, vs], func=AF.Exp)
                                    # accumulate p on vector (k-partial rowsums)
                                    if kt == 0:
                                        nc.vector.tensor_copy(psum_v, p_t)
                                    else:
                                        nc.vector.tensor_add(
                                            psum_v[:, vs], psum_v[:, vs], p_t[:, vs]
                                        )
                                    nc.tensor.matmul(
                                        op[:, vs],
                                        v_sb[h][:, ks],
                                        p_t[:, vs],
                                        start=(kt == 0), stop=(kt == nkt - 1),
                                    )
                                pcast = psa.tile([128, NQ], BF, tag="pcast")
                                nc.vector.tensor_copy(pcast, psum_v)
                                nc.tensor.matmul(rs, ones_t, pcast, start=True, stop=True)
                                rv = rvp.tile([128, NQ], F32, tag="rv")
                                nc.vector.reciprocal(rv, rs)
                                nc.vector.tensor_mul(o_T[h][:, cs], op, rv)
                            # o_proj for the PREVIOUS chunk (o_T long ready)
                            if c >= 1:
                                for si in range(4 * (c - 1), 4 * c):
                                    oproj(ostg, out_ps, si)
                    # ---------------- final chunk o_proj ----------------
                    with tc.tile_pool(name="ostg2", bufs=4) as ostg2, \
                         tc.tile_pool(name="out2_ps", bufs=4, space="PSUM") as out2_ps:
                        for si in range(4 * (NCHUNK - 1), 4 * NCHUNK):
                            oproj(ostg2, out2_ps, si)
            wc_cm.__exit__(None, None, None)
    _split_waits(nc)
    return nc


# ----------------------------------------------------------------------------
# entry point
# ----------------------------------------------------------------------------

def kernel(**inputs):
    global LAST_RESULTS
    shared, per_core = _prep_inputs(inputs)
    if "nc" not in _CACHE:
        _CACHE["nc"] = _build_nc()
    nc = _CACHE["nc"]
    in_maps = []
    for c in range(N_CORES):
        m = {
            "qaT": shared["qaT"],
            "kvaT": shared["kvaT"],
            "qbAllT": shared["qbAllT"],
            "maskb": shared["maskb"],
            "hTloc": per_core[c]["hTloc"],
            "cosl": per_core[c]["cosl"],
            "sinl": per_core[c]["sinl"],
            "kbT": per_core[c]["kbT"],
            "vbT": per_core[c]["vbT"],
            "owT": per_core[c]["owT"],
        }
        in_maps.append(m)
    res = run_bass_kernel_spmd(nc, in_maps, core_ids=list(range(N_CORES)))
    LAST_RESULTS = res
    out = np.zeros((S, D), dtype=np.float32)
    for r in res.results:
        out += np.asarray(r["out"], dtype=np.float32)
    return out.reshape(B, S, D)
